# revision 10
# baseline (speedup 1.0000x reference)
"""Trainium2 Bass kernel for nn_Cross_LocalAttention (windowed local attention + LePE).

Sharding: data-parallel over batch B=8 -> one batch per NeuronCore (8 cores).
Per core: q,k,v [16384, 256] f32; outputs x [16384, 256] and q_out [256, 8, 64, 32].

Per-core algorithm (window grid 16x16, window 8x8=64 tokens, 8 heads, hd=32):
  - process "quads" of 4 windows (2 window-rows x 2 window-cols):
      natural tiles [128 part = (ph, r, s), 512 free = (jj, c)]
  - qs = q*SCALE (f32, also DMAed out as q_out), bf16 copies of qs/k/v
  - PE transposes -> QT/KT (d-major, [128 c-half, 4win x 64t]) and VTpad (10x10
    zero-padded grids for the depthwise conv)
  - scores^T per window-head via PE-tiled matmuls (K=32 quadrants), exp on ACT
  - AV: lhsT=exp(S^T) stationary, rhs = bf16 V interleaved with a ones column
    -> token-major unnormalized out + denominator in PSUM
  - LePE: 9-tap depthwise conv via scalar_tensor_tensor FMAs in [c, t] layout,
    transposed back to [t, c] with a full 128x128 PE transpose
  - final: x = (AV * 1/den) + lepe  (DVE), DMA out
"""

import os
import sys
import threading

import numpy as np

sys.path.insert(0, "/opt/trn_rl_repo")

import concourse.bass as bass
import concourse.mybir as mybir
import concourse.tile as tile
from concourse import bacc
from concourse.masks import make_identity

F32 = mybir.dt.float32
BF16 = mybir.dt.bfloat16

B, H, W, DIM, HEADS = 8, 128, 128, 256, 8
LH, LW = 8, 8
HD = DIM // HEADS
SCALE = HD ** -0.5
WS = LH * LW
N_CORES = 8


def _ap_with_dims(t_ap, offset_elems, dims):
    """Build a raw AP view over a tile's underlying tensor: dims = [[step, count], ...]
    (first dim = partitions)."""
    import dataclasses

    new = dataclasses.replace(
        t_ap, offset=t_ap.offset + offset_elems, ap=[list(d) for d in dims]
    )
    return new


def build_program(n_wr=16, n_wc=16, num_devices=N_CORES, stages=5):
    """Build the per-core program for an n_wr x n_wc window grid."""
    nc = bacc.Bacc(
        "TRN2",
        target_bir_lowering=False,
        debug=False,
        enable_asserts=False,
        num_devices=num_devices,
    )
    Hl, Wl = 8 * n_wr, 8 * n_wc
    L = Hl * Wl
    NWIN = n_wr * n_wc

    t_q = nc.dram_tensor("q", [L, DIM], F32, kind="ExternalInput")
    t_k = nc.dram_tensor("k", [L, DIM], F32, kind="ExternalInput")
    t_v = nc.dram_tensor("v", [L, DIM], F32, kind="ExternalInput")
    t_w = nc.dram_tensor("lepe_w", [DIM, 9], F32, kind="ExternalInput")
    t_b = nc.dram_tensor("lepe_b", [DIM], F32, kind="ExternalInput")
    t_x = nc.dram_tensor("x", [L, DIM], F32, kind="ExternalOutput")
    t_qo = nc.dram_tensor("qout", [NWIN, HEADS, WS, HD], F32, kind="ExternalOutput")

    # DRAM views
    # token L = (wi*8 + r)*Wl + wj*8 + s
    qr = t_q.ap().rearrange("(wi r wj s) c -> wi r s wj c", wi=n_wr, r=8, wj=n_wc, s=8)
    kr = t_k.ap().rearrange("(wi r wj s) c -> wi r s wj c", wi=n_wr, r=8, wj=n_wc, s=8)
    vr = t_v.ap().rearrange("(wi r wj s) c -> wi r s wj c", wi=n_wr, r=8, wj=n_wc, s=8)
    xr = t_x.ap().rearrange("(wi r wj s) c -> wi r s wj c", wi=n_wr, r=8, wj=n_wc, s=8)
    # qout[win, h, t, d] with win = wi*n_wc + wj; src partition = (ph, t)
    qor = t_qo.ap().rearrange("(wi wj) h t d -> wi t wj h d", wi=n_wr)
    wr_ = t_w.ap().rearrange("(h2 c) k -> c h2 k", h2=2)
    br_ = t_b.ap().rearrange("(h2 c) -> c h2", h2=2)

    with tile.TileContext(nc) as tc:
        _build_tiles(tc, qr, kr, vr, xr, qor, wr_, br_, n_wr, n_wc, stages)
    nc.compile()
    return nc


def _build_tiles(tc, qr, kr, vr, xr, qor, wr_, br_, n_wr, n_wc, stages=5):
    from contextlib import ExitStack

    nc = tc.nc
    ctx = ExitStack()
    with ctx:
        const_p = ctx.enter_context(tc.tile_pool(name="const", bufs=1))
        load_p = ctx.enter_context(tc.tile_pool(name="load", bufs=3))
        qs_p = ctx.enter_context(tc.tile_pool(name="qs", bufs=3))
        qt_p = ctx.enter_context(tc.tile_pool(name="qt", bufs=4))
        vtp_p = ctx.enter_context(tc.tile_pool(name="vtp", bufs=8))
        exp_p = ctx.enter_context(tc.tile_pool(name="expp", bufs=8))
        vint_p = ctx.enter_context(tc.tile_pool(name="vint", bufs=3))
        acc_p = ctx.enter_context(tc.tile_pool(name="accp", bufs=8))
        small_p = ctx.enter_context(tc.tile_pool(name="small", bufs=8))
        xout_p = ctx.enter_context(tc.tile_pool(name="xout", bufs=3))
        ps_tp = ctx.enter_context(tc.tile_pool(name="ps_tp", bufs=2, space="PSUM"))
        ps_sc = ctx.enter_context(tc.tile_pool(name="ps_sc", bufs=2, space="PSUM"))
        ps_av = ctx.enter_context(tc.tile_pool(name="ps_av", bufs=4, space="PSUM"))

        # constants
        identf = const_p.tile([128, 128], F32, tag="identf")
        make_identity(nc, identf[:, :])
        identb = const_p.tile([128, 128], BF16, tag="identb")
        make_identity(nc, identb[:, :])
        wtile = const_p.tile([128, 18], F32, tag="wtile")
        nc.sync.dma_start(wtile[:, :], wr_)
        btile = const_p.tile([128, 2], F32, tag="btile")
        nc.sync.dma_start(btile[:, :], br_)
        zt = const_p.tile([128, 64], F32, tag="zt")
        nc.gpsimd.memset(zt[:, :], 0.0)
        bias_bc = const_p.tile([128, 128], F32, tag="bias_bc")
        for h2 in range(2):
            nc.scalar.activation(
                bias_bc[:, 64 * h2 : 64 * (h2 + 1)],
                zt[:, :],
                mybir.ActivationFunctionType.Identity,
                bias=btile[:, h2 : h2 + 1],
                scale=1.0,
            )

        TAPS = [(0, 0)] + [
            (dy, dx) for dy in (-1, 0, 1) for dx in (-1, 0, 1) if (dy, dx) != (0, 0)
        ]

        for i2 in range(0, n_wr, 2):
            for j2 in range(0, n_wc, 2):
                # ---- loads ----
                qf = load_p.tile([128, 512], F32, tag="qf")
                kf = load_p.tile([128, 512], F32, tag="kf")
                vf = load_p.tile([128, 512], F32, tag="vf")
                for jj in range(2):
                    fs = slice(256 * jj, 256 * (jj + 1))
                    nc.sync.dma_start(qf[:, fs], qr[i2 : i2 + 2, :, :, j2 + jj, :])
                    nc.sync.dma_start(kf[:, fs], kr[i2 : i2 + 2, :, :, j2 + jj, :])
                    nc.sync.dma_start(vf[:, fs], vr[i2 : i2 + 2, :, :, j2 + jj, :])

                # scaled f32 q for the qout output (scale for scores is folded
                # into the exp activation instead)
                qs = qs_p.tile([128, 512], F32, tag="qs")
                nc.gpsimd.tensor_scalar_mul(qs[:, :], qf[:, :], SCALE)
                for jj in range(2):
                    for ph in range(2):
                        nc.sync.dma_start(
                            qor[i2 + ph, :, j2 + jj, :, :],
                            qs[64 * ph : 64 * (ph + 1), 256 * jj : 256 * (jj + 1)],
                        )

                # ---- PE transposes (one per PSUM tile; full 128x128, f32) ----
                # out: [128 c-half, 128 = (ph0 64t | ph1 64t)] per (tensor, h2, jj)
                qt = {}
                kt = {}
                vtp = {}
                for h2 in range(2):
                    qt[h2] = qt_p.tile([128, 256], BF16, tag=f"qt{h2}", name=f"qt{h2}")
                    kt[h2] = qt_p.tile([128, 256], BF16, tag=f"kt{h2}", name=f"kt{h2}")
                for jj in range(2):
                    for h2 in range(2):
                        cs = slice(256 * jj + 128 * h2, 256 * jj + 128 * (h2 + 1))
                        tq = ps_tp.tile([128, 128], F32, tag="tp", name="tpq")
                        nc.tensor.transpose(tq[:, :], qf[:, cs], identf[:, :])
                        nc.scalar.copy(qt[h2][:, 128 * jj : 128 * (jj + 1)], tq[:, :])
                        tk = ps_tp.tile([128, 128], F32, tag="tp", name="tpk")
                        nc.tensor.transpose(tk[:, :], kf[:, cs], identf[:, :])
                        nc.scalar.copy(kt[h2][:, 128 * jj : 128 * (jj + 1)], tk[:, :])
                        tv = ps_tp.tile([128, 128], F32, tag="tp", name="tpv")
                        nc.tensor.transpose(tv[:, :], vf[:, cs], identf[:, :])
                        t = vtp_p.tile([128, 200], BF16, tag="vtp", name="vtp")
                        vtp[(jj, h2)] = t
                        nc.gpsimd.memset(t[:, :], 0.0)
                        for ph in range(2):
                            dst = t[:, 100 * ph : 100 * (ph + 1)].rearrange(
                                "p (yy xx) -> p yy xx", yy=10
                            )
                            nc.scalar.copy(
                                dst[:, 1:9, 1:9], tv[:, 64 * ph : 64 * (ph + 1)]
                            )

                # ---- scores: one PSUM bank per row-quadrant r ----
                scps = {}
                for r4 in range(4):
                    scps[r4] = ps_sc.tile([128, 512], F32, tag="scps", name="scps")
                for jj in range(2):
                    for ph in range(2):
                        w = 2 * jj + ph
                        for h in range(HEADS):
                            h2, r4 = h // 4, h % 4
                            r = 32 * r4
                            wcol = slice(128 * jj + 64 * ph, 128 * jj + 64 * (ph + 1))
                            nc.tensor.matmul(
                                scps[r4][
                                    64 * ph : 64 * (ph + 1),
                                    256 * h2 + 64 * w : 256 * h2 + 64 * (w + 1),
                                ],
                                kt[h2][r : r + 32, wcol],
                                qt[h2][r : r + 32, wcol],
                                start=True,
                                stop=True,
                                tile_position=(r, 64 * ph),
                            )

                # ---- exp (folds the 1/sqrt(d) scale) ----
                expt = {}
                for r4 in range(4):
                    e = exp_p.tile([128, 512], BF16, tag="expt", name="expt")
                    expt[r4] = e
                    for ph in range(2):
                        sc_ap = _ap_with_dims(
                            scps[r4][64 * ph : 64 * (ph + 1), :],
                            64 * ph,
                            [list(scps[r4][64 * ph : 64 * (ph + 1), :].ap[0]), [128, 4], [1, 64]],
                        )
                        e_ap = _ap_with_dims(
                            e[64 * ph : 64 * (ph + 1), :],
                            64 * ph,
                            [list(e[64 * ph : 64 * (ph + 1), :].ap[0]), [128, 4], [1, 64]],
                        )
                        nc.scalar.activation(
                            e_ap,
                            sc_ap,
                            mybir.ActivationFunctionType.Exp,
                            scale=float(SCALE),
                        )

                # ---- V interleaved with ones column (bf16, cast from f32) ----
                vint = vint_p.tile([128, 528], BF16, tag="vint")
                for jj in range(2):
                    vi_ap = vint[:, 264 * jj : 264 * (jj + 1)].rearrange(
                        "p (h e) -> p h e", h=8
                    )
                    nc.gpsimd.tensor_copy(
                        vi_ap[:, :, 0:32],
                        vf[:, 256 * jj : 256 * (jj + 1)].rearrange(
                            "p (h d) -> p h d", h=8
                        ),
                    )
                    nc.gpsimd.memset(vi_ap[:, :, 32:33], 1.0)

                # ---- AV: one PSUM bank per (jj, ph) ----
                avps = {}
                for jj in range(2):
                    for ph in range(2):
                        av = ps_av.tile([128, 264], F32, tag="avps", name="avps")
                        avps[(jj, ph)] = av
                        w = 2 * jj + ph
                        for h in range(HEADS):
                            h2, r4 = h // 4, h % 4
                            nc.tensor.matmul(
                                av[64 * ph : 64 * (ph + 1), 33 * h : 33 * (h + 1)],
                                expt[r4][
                                    64 * ph : 64 * (ph + 1),
                                    256 * h2 + 64 * w : 256 * h2 + 64 * (w + 1),
                                ],
                                vint[
                                    64 * ph : 64 * (ph + 1),
                                    33 * (8 * jj + h) : 33 * (8 * jj + h) + 33,
                                ],
                                start=True,
                                stop=True,
                                tile_position=(64 * ph, 64 * ph),
                            )

                # ---- LePE taps (DVE h2=0, GPSIMD h2=1) ----
                accs = {}
                for jj in range(2):
                    for h2 in range(2):
                        acc = acc_p.tile([128, 128], BF16, tag="accp", name="accp")
                        accs[(jj, h2)] = acc
                        eng = nc.vector
                        for ph in range(2):
                            acc_v = acc[:, 64 * ph : 64 * (ph + 1)].rearrange(
                                "p (y x) -> p y x", y=8
                            )
                            vt_g = vtp[(jj, h2)][
                                :, 100 * ph : 100 * (ph + 1)
                            ].rearrange("p (yy xx) -> p yy xx", yy=10)
                            first = True
                            for dy, dx in TAPS:
                                ktap = 3 * (dy + 1) + (dx + 1)
                                in0 = vt_g[:, 1 + dy : 9 + dy, 1 + dx : 9 + dx]
                                in1 = (
                                    bias_bc[:, 64 * h2 : 64 * h2 + 64].rearrange(
                                        "p (y x) -> p y x", y=8
                                    )
                                    if first
                                    else acc_v
                                )
                                eng.scalar_tensor_tensor(
                                    acc_v,
                                    in0,
                                    wtile[:, 9 * h2 + ktap : 9 * h2 + ktap + 1],
                                    in1,
                                    op0=mybir.AluOpType.mult,
                                    op1=mybir.AluOpType.add,
                                )
                                first = False

                # ---- LePE transpose back to [t, c] (one per PSUM tile) ----
                lepeps = {}
                for jj in range(2):
                    for h2 in range(2):
                        lp = ps_tp.tile([128, 128], BF16, tag="tp", name="lepeps")
                        lepeps[(jj, h2)] = lp
                        nc.tensor.transpose(
                            lp[:, :], accs[(jj, h2)][:, :], identb[:, :]
                        )

                # ---- normalize + combine + store ----
                xq = xout_p.tile([128, 512], F32, tag="xq")
                for jj in range(2):
                    for ph in range(2):
                        av = avps[(jj, ph)]
                        pslice = slice(64 * ph, 64 * (ph + 1))
                        rcp = small_p.tile([128, 8], F32, tag="rcp", name="rcp")
                        den_ap = _ap_with_dims(
                            av[pslice, :], 32, [list(av[pslice, :].ap[0]), [33, 8]]
                        )
                        nc.vector.reciprocal(rcp[pslice, 0:8], den_ap)
                        av_v = _ap_with_dims(
                            av[pslice, :], 0, [list(av[pslice, :].ap[0]), [33, 8], [1, 32]]
                        )
                        rbc_src = _ap_with_dims(
                            rcp[pslice, 0:8], 0, [list(rcp[pslice, 0:8].ap[0]), [1, 8], [0, 32]]
                        )
                        t1 = small_p.tile([128, 256], F32, tag="t1", name="t1")
                        nc.vector.tensor_tensor(
                            t1[pslice, :].rearrange("p (h d) -> p h d", h=8),
                            av_v,
                            rbc_src,
                            op=mybir.AluOpType.mult,
                        )
                        for h2 in range(2):
                            nc.vector.tensor_add(
                                xq[pslice, 256 * jj + 128 * h2 : 256 * jj + 128 * (h2 + 1)],
                                t1[pslice, 128 * h2 : 128 * (h2 + 1)],
                                lepeps[(jj, h2)][pslice, :],
                            )
                for jj in range(2):
                    nc.sync.dma_start(
                        xr[i2 : i2 + 2, :, :, j2 + jj, :],
                        xq[:, 256 * jj : 256 * (jj + 1)],
                    )


_BUILD_LOCK = threading.Lock()
_CACHED = {}


def _get_program(n_wr=16, n_wc=16):
    key = (n_wr, n_wc)
    with _BUILD_LOCK:
        if key not in _CACHED:
            _CACHED[key] = build_program(n_wr, n_wc)
    return _CACHED[key]


def kernel(qkv, lepe_w, lepe_b, _trace=False, _trace_kwargs=None):
    qkv = np.asarray(qkv, dtype=np.float32)
    lepe_w = np.asarray(lepe_w, dtype=np.float32)
    lepe_b = np.asarray(lepe_b, dtype=np.float32)
    w9 = np.ascontiguousarray(lepe_w.reshape(DIM, 9))
    nc = _get_program()

    in_maps = []
    for c in range(N_CORES):
        in_maps.append(
            {
                "q": np.ascontiguousarray(qkv[0, c]),
                "k": np.ascontiguousarray(qkv[1, c]),
                "v": np.ascontiguousarray(qkv[2, c]),
                "lepe_w": w9,
                "lepe_b": lepe_b,
            }
        )

    from concourse.bass_utils import run_bass_kernel_spmd

    kwargs = {}
    if _trace:
        kwargs["trace"] = True
        if _trace_kwargs:
            kwargs.update(_trace_kwargs)
    res = run_bass_kernel_spmd(nc, in_maps, core_ids=list(range(N_CORES)), **kwargs)
    x = np.stack([res.results[c]["x"] for c in range(N_CORES)], axis=0)
    qout = np.concatenate([res.results[c]["qout"] for c in range(N_CORES)], axis=0)
    if _trace:
        return (x, qout), res
    return x, qout


if __name__ == "__main__":
    # quick smoke build
    nc = build_program(2, 2, num_devices=1)
    print("built ok:", len(nc.m.functions[0].instructions) if hasattr(nc.m.functions[0], "instructions") else "?")
